# revision 68
# baseline (speedup 1.0000x reference)
"""Multi-head attention kernel for Trainium2, SPMD over 8 NeuronCores.

Sharding: 2(batch) x 2(k-half) x 2(head-half). Each core holds one batch's
k/v slice of 4096 rows and computes K/V/Q projections + masked-softmax
attention for its 4 local heads, entirely independently — NO collectives.
For every head it ships the *unnormalized* attention numerator projected
through that head's Wf rows ([512, 1024] f32 partial) plus the softmax
denominators; the host sums each k-half pair's partials and denominators
and divides (the standard softmax shard combine — projection commutes
with the k-half sum because the per-q denominator is a scalar). This
trades ~4MB of extra output DMA (hidden under compute) for the ~20-35us
serialized ReduceScatter latencies that previously dominated the tail.

Layout notes: all activations/weights/mask pre-transposed and pre-cast to
bf16 on the host; scores computed transposed ([k, q]) so the exp output is
directly the stationary operand of the AV matmul; multiplicative bf16 mask
after exp; softmax denominator rides as a 129th v-column through AV; no
max-subtraction (scores are O(1)).

Engine schedule: scalar (ACT) runs only exp (plus a few DMA triggers);
vector owns PSUM->SBUF copies and half the mask multiplies (gpsimd takes
the other half — it has no collectives to run); head h's numerators are
transposed (PE), projected through Wf_h (PE), staged and DMA'd out right
after head h, so head h+1's compute hides the shipping. Head 0's probs
are precomputed during the V projection (ACT idle there); the attention
loop scores head s+1 while accumulating head s. Startup DMAs are split
into 256-512KB pieces; the xk stream's later chunks ride the scalar queue
once the weights are down, keeping the K projection fed.
"""

import sys

if "/opt/trn_rl_repo" not in sys.path:
    sys.path.insert(0, "/opt/trn_rl_repo")

from contextlib import ExitStack

import ml_dtypes
import numpy as np

import concourse.bass as bass  # noqa: F401
import concourse.mybir as mybir
import concourse.tile as tile
from concourse import bacc
from concourse.masks import make_identity

B, QL, KL, D, H = 2, 512, 8192, 1024, 8
HD = D // H  # 128
NCORES = 8
KSH = KL // 2  # 4096 k rows per core
HL = 4  # local heads per core
SCALE = 1.0 / float(np.sqrt(HD))

F32 = mybir.dt.float32
BF16 = mybir.dt.bfloat16
P = 128
KC = KSH // P  # 32 k chunks of 128
QB = QL // P  # 4 q blocks
DB = D // P  # 8 d-in blocks
NCH = KSH // 512  # 8 streaming chunks of 512 k rows


def ensure_ntff_hook():
    """Provide antenv.axon_hooks (missing in this image) so trace=True works.

    Mirrors trn_agent_boot._ntff_profile_via_ctypes against the local
    libaxon_pjrt.so. No-op if the real module exists or the .so is absent.
    """
    try:
        import antenv.axon_hooks  # noqa: F401

        return
    except ImportError:
        pass
    import contextlib
    import ctypes
    import types

    mod = types.ModuleType("antenv.axon_hooks")
    holder = [None]
    mod.set_axon_ntff_profile_hook = lambda h: holder.__setitem__(0, h)
    mod.get_axon_ntff_profile_hook = lambda: holder[0]
    try:
        lib = ctypes.CDLL("/opt/axon/libaxon_pjrt.so")
        if hasattr(lib, "axon_start_nrt_profile"):
            lib.axon_start_nrt_profile.argtypes = [
                ctypes.POINTER(ctypes.c_int64),
                ctypes.c_size_t,
            ]
            lib.axon_start_nrt_profile.restype = ctypes.c_int64
            lib.axon_stop_nrt_profile.argtypes = [ctypes.c_char_p]
            lib.axon_stop_nrt_profile.restype = ctypes.c_int64

            @contextlib.contextmanager
            def _hook(output_dir, device_ids):
                import jax

                jax.devices()
                if device_ids:
                    ids = (ctypes.c_int64 * len(device_ids))(*device_ids)
                    rc = lib.axon_start_nrt_profile(ids, len(device_ids))
                else:
                    rc = lib.axon_start_nrt_profile(None, 0)
                if rc != 0:
                    raise RuntimeError(f"axon_start_nrt_profile rc={rc}")
                try:
                    yield
                finally:
                    n = lib.axon_stop_nrt_profile(str(output_dir).encode())
                    print(f"ntff profile: {n} file(s) -> {output_dir}")

            holder[0] = _hook
    except OSError:
        pass
    sys.modules["antenv.axon_hooks"] = mod
    try:
        import antenv

        antenv.axon_hooks = mod
    except ImportError:
        pass


def build_attention_kernel():
    nc = bacc.Bacc(
        "TRN2", target_bir_lowering=False, debug=False, num_devices=NCORES
    )

    xqT = nc.declare_dram_parameter("xqT", [D, QL], BF16, isOutput=False)
    xkT = nc.declare_dram_parameter("xkT", [D, KSH], BF16, isOutput=False)
    xvT = nc.declare_dram_parameter("xvT", [D, KSH], BF16, isOutput=False)
    mskT = nc.declare_dram_parameter("mskT", [KSH, QL], BF16, isOutput=False)
    wqT = nc.declare_dram_parameter("wqT", [D, HL * HD], BF16, isOutput=False)
    wkT = nc.declare_dram_parameter("wkT", [D, HL * HD], BF16, isOutput=False)
    wvT = nc.declare_dram_parameter("wvT", [D, HL * HD], BF16, isOutput=False)
    wfT = nc.declare_dram_parameter("wfT", [HL * HD, D], BF16, isOutput=False)
    # per-head unnormalized projected partials + softmax denominators;
    # the host sums each k-half pair and divides (softmax shard combine).
    outp = nc.declare_dram_parameter("outp", [HL, QB, P, D], BF16, isOutput=True)
    dnum = nc.declare_dram_parameter("dnum", [HL, QB, P], F32, isOutput=True)

    with tile.TileContext(nc) as tc, ExitStack() as ctx:
        # Persistent operand tiles (single-buffered, live for the kernel).
        persist = ctx.enter_context(tc.tile_pool(name="persist", bufs=1))
        kT = persist.tile([P, HL, KSH], BF16)  # [hd, head, krow]
        v_sb = persist.tile([P, KC, HL, HD + 1], BF16)  # [krow, kc, h, hd+1]
        mask_sb = persist.tile([P, KC, QL], BF16)  # [k, kc, q]
        qT = persist.tile([P, HL, QL], BF16)  # [hd, head, q]
        wv_sb = persist.tile([P, DB, HL * HD], BF16)
        wf_sb = persist.tile([P, HL, D], BF16)  # [hd, head, dout]
        numT = persist.tile([P, 2, QB, P], BF16)  # numerators^T, 2 in flight
        dn_sb = persist.tile([P, HL, QB], F32)  # denominators

        # 32KB alias block: wk/wq/xq early, head-0 precomputed probs late.
        # (wk dies at K-proj end, wq/xq after the Q projection; pp0's first
        # write happens during the V projection, strictly later.)
        ablk = persist.tile([P, KC, QL], BF16, name="ablk")
        wk_sb = ablk[:, 0:DB, :]  # [P, 8, 512]
        wq_sb = ablk[:, DB : 2 * DB, :]
        xq_sb = ablk[:, 2 * DB : 3 * DB, :]
        pp0 = ablk  # [P, KC, QL] head-0 probs, precomputed

        loads = ctx.enter_context(tc.tile_pool(name="loads", bufs=3))
        probs_pool = ctx.enter_context(tc.tile_pool(name="probs", bufs=8))
        nums = ctx.enter_context(tc.tile_pool(name="nums", bufs=2))
        stg = ctx.enter_context(tc.tile_pool(name="stg", bufs=6))

        consts = ctx.enter_context(tc.tile_pool(name="consts", bufs=1))
        ident = consts.tile([P, P], BF16)
        gate_t = consts.tile([P, 2], BF16)
        make_identity(nc, ident)

        # One PSUM pool, 8 banks: mm 2x2 + av 4x1.
        psum = ctx.enter_context(tc.tile_pool(name="psum", bufs=1, space="PSUM"))

        def mm_tile(name, dtype=F32):
            return psum.tile([P, 2, 512], dtype, tag="mm", bufs=2, name=name)

        def av_tile(name, cols=HD + 1):
            return psum.tile([P, cols], F32, tag="av", bufs=4, name=name)

        # --- DMA loads, split into ~256-512KB pieces on three queues so the
        # first K-proj matmul can start ~1.5us in.
        #   sync:   xk chunks 0,1,2,4,6
        #   scalar: wk first (parallel with xk), wq, xq, wf, xk chunks 3,5,7
        #   gpsimd: mask, wv, then the xv stream
        def load_pair(eng, dst, src, lo, hi):
            eng.dma_start(
                out=dst[:, lo // P : hi // P, :],
                in_=src[lo:hi, :].rearrange("(a p) d -> p a d", p=P),
            )

        def load_xchunk(eng, dst, src, c, half):
            lo, hi = half * 512, (half + 1) * 512
            eng.dma_start(
                out=dst[:, lo // P : hi // P, :],
                in_=src[lo:hi, c * 512 : (c + 1) * 512].rearrange(
                    "(a p) k -> p a k", p=P
                ),
            )

        for i in range(4):
            load_pair(nc.scalar, wk_sb, wkT, i * 256, (i + 1) * 256)
        # chunk 0 split into two 256-col sub-chunks of 2x256KB pieces so
        # the first matmuls start as soon as ~0.75MB has landed.
        xk0s = []
        for sub in range(2):
            t = loads.tile([P, DB, 256], BF16, tag="ld", name=f"xk0s{sub}")
            for half in range(2):
                lo = half * 512
                nc.sync.dma_start(
                    out=t[:, lo // P : (lo + 512) // P, :],
                    in_=xkT[lo : lo + 512, sub * 256 : (sub + 1) * 256].rearrange(
                        "(a p) k -> p a k", p=P
                    ),
                )
            xk0s.append(t)
        # chunk 1 rides the gpsimd queue (idle until the gate opens) so it
        # lands in parallel with chunk 0 instead of queueing behind it.
        xkc1 = loads.tile([P, DB, 512], BF16, tag="ld", name="xkc1")
        for half in range(2):
            load_xchunk(nc.gpsimd, xkc1, xkT, 1, half)

        load_pair(nc.scalar, wq_sb, wqT, 0, 512)
        load_pair(nc.scalar, wq_sb, wqT, 512, 1024)
        load_pair(nc.scalar, xq_sb, xqT, 0, 512)
        load_pair(nc.scalar, xq_sb, xqT, 512, 1024)
        nc.scalar.dma_start(
            out=wf_sb, in_=wfT.rearrange("(i p) d -> p i d", p=P)
        )

        # --- K projection: 8 chunks of 512 k rows; 2 head-pairs each.
        def k_proj_chunk(c, xkc):
            for hp in range(2):
                pk = mm_tile(f"pk_{c}_{hp}")
                for i in range(2):
                    for a in range(DB):
                        nc.tensor.matmul(
                            pk[:, i, :],
                            wk_sb[:, a, hp * 256 + i * HD : hp * 256 + (i + 1) * HD],
                            xkc[:, a, :],
                            start=(a == 0),
                            stop=(a == DB - 1),
                        )
                nc.vector.tensor_copy(
                    out=kT[:, 2 * hp : 2 * hp + 2, c * 512 : (c + 1) * 512],
                    in_=pk[:],
                )

        xk_eng = {3: nc.scalar, 5: nc.scalar, 7: nc.scalar}

        # chunk 0 via the two 256-col sub-chunks
        for sub in range(2):
            for hp in range(2):
                pk = mm_tile(f"pk0_{sub}_{hp}")
                for i in range(2):
                    for a in range(DB):
                        nc.tensor.matmul(
                            pk[:, i, 0:256],
                            wk_sb[:, a, hp * 256 + i * HD : hp * 256 + (i + 1) * HD],
                            xk0s[sub][:, a, :],
                            start=(a == 0),
                            stop=(a == DB - 1),
                        )
                nc.vector.tensor_copy(
                    out=kT[
                        :, 2 * hp : 2 * hp + 2, sub * 256 : (sub + 1) * 256
                    ],
                    in_=pk[:, :, 0:256],
                )
        xkc2 = loads.tile([P, DB, 512], BF16, tag="ld", name="xkc2")
        for half in range(2):
            load_xchunk(nc.sync, xkc2, xkT, 2, half)
        # gate the gpsimd stream (mask/wv/xv — none needed before the V
        # phase) behind xkc2's arrival so the K path gets the full early
        # HBM bandwidth: a 2-element copy creates the queue dependency.
        nc.gpsimd.tensor_copy(out=gate_t[:], in_=xkc2[:, 0, 0:2])
        for i in range(4):
            nc.gpsimd.dma_start(
                out=mask_sb[:, i * 8 : (i + 1) * 8, :],
                in_=mskT[i * 1024 : (i + 1) * 1024, :].rearrange(
                    "(a p) q -> p a q", p=P
                ),
            )
        load_pair(nc.gpsimd, wv_sb, wvT, 0, 512)
        load_pair(nc.gpsimd, wv_sb, wvT, 512, 1024)
        k_proj_chunk(1, xkc1)
        xkc3 = loads.tile([P, DB, 512], BF16, tag="ld", name="xkc3")
        for half in range(2):
            load_xchunk(xk_eng.get(3, nc.sync), xkc3, xkT, 3, half)
        k_proj_chunk(2, xkc2)

        # --- Q projection for this core's 4 heads (local; no AllGather).
        # Placed here so its wq/xq DMAs (behind wk on scalar) have landed.
        for hp in range(2):
            pq = mm_tile(f"pq_{hp}")
            for i in range(2):
                for a in range(DB):
                    nc.tensor.matmul(
                        pq[:, i, :],
                        wq_sb[:, a, hp * 256 + i * HD : hp * 256 + (i + 1) * HD],
                        xq_sb[:, a, :],
                        start=(a == 0),
                        stop=(a == DB - 1),
                    )
            nc.vector.tensor_copy(out=qT[:, 2 * hp : 2 * hp + 2, :], in_=pq[:])

        nxt = xkc3
        for c in range(3, NCH):
            if c < NCH - 1:
                nxtc = loads.tile([P, DB, 512], BF16, tag="ld", name=f"xkc{c + 1}")
                for half in range(2):
                    load_xchunk(xk_eng.get(c + 1, nc.sync), nxtc, xkT, c + 1, half)
            k_proj_chunk(c, nxt)
            nxt = nxtc if c < NCH - 1 else None

        # --- V projection (xvT streamed); head-0 probs precomputed alongside
        # (ACT is otherwise idle here). One pre_probs per (c, mkl).
        def pre_probs(kc):
            ps = av_tile(f"pps_{kc}", 512)
            nc.tensor.matmul(
                ps[:],
                kT[:, 0, kc * P : (kc + 1) * P],
                qT[:, 0, :],
                start=True,
                stop=True,
            )
            nc.scalar.activation(
                pp0[:, kc, :], ps[:], mybir.ActivationFunctionType.Exp, scale=SCALE
            )
            nc.vector.tensor_mul(
                pp0[:, kc, :], pp0[:, kc, :], mask_sb[:, kc, :]
            )

        xvc_next = loads.tile([P, DB, 512], BF16, tag="ld", name="xvc0")
        for half in range(2):
            load_xchunk(nc.gpsimd, xvc_next, xvT, 0, half)
        for c in range(NCH):
            xvc = xvc_next
            if c < NCH - 1:
                xvc_next = loads.tile(
                    [P, DB, 512], BF16, tag="ld", name=f"xvc{c + 1}"
                )
                for half in range(2):
                    load_xchunk(nc.gpsimd, xvc_next, xvT, c + 1, half)
            for mkl in range(4):
                mk = c * 4 + mkl
                pv = av_tile(f"pv_{mk}", 512)
                for a in range(DB):
                    nc.tensor.matmul(
                        pv[:],
                        xvc[:, a, mkl * P : (mkl + 1) * P],
                        wv_sb[:, a, :],
                        start=(a == 0),
                        stop=(a == DB - 1),
                    )
                nc.vector.tensor_copy(
                    out=v_sb[:, mk, :, 0:HD],
                    in_=pv[:].rearrange("p (b c) -> p b c", b=HL),
                )
                pre_probs(mk)
        nc.vector.memset(v_sb[:, :, :, HD], 1.0)

        # --- ship head h: transpose the unnormalized numerator, project it
        # through this head's Wf rows, DMA the [512, 1024] partial + denoms.
        # No cross-core communication; the host combines the k-half pair.
        # Emitted as 5 pieces so the work interleaves with the NEXT head's
        # j-loop instead of serializing at the head boundary.
        def ship_pieces(h, num):
            def p_transpose():
                nc.vector.tensor_copy(out=dn_sb[:, h, :], in_=num[:, :, HD])
                pstn = mm_tile(f"pstn_{h}", BF16)
                for qb in range(QB):
                    nc.tensor.transpose(
                        pstn[:, qb // 2, (qb % 2) * P : (qb % 2 + 1) * P],
                        num[:, qb, 0:HD],
                        ident,
                    )
                nT = numT[:, h % 2]
                for a in range(2):
                    nc.vector.tensor_copy(
                        out=nT[:, 2 * a : 2 * a + 2, :],
                        in_=pstn[:, a, 0 : 2 * P].rearrange(
                            "p (b c) -> p b c", b=2
                        ),
                    )
                # scalar runs exps while heads 0-2 ship; route via sync.
                dn_eng = nc.scalar if h == HL - 1 else nc.sync
                dn_eng.dma_start(
                    out=dnum[h].rearrange("q p -> p q"), in_=dn_sb[:, h, :]
                )

            def p_proj(qb):
                def f():
                    nT = numT[:, h % 2]
                    po = mm_tile(f"po_{h}_{qb}")
                    for n in range(2):
                        nc.tensor.matmul(
                            po[:, n, :],
                            nT[:, qb, :],
                            wf_sb[:, h, n * 512 : (n + 1) * 512],
                            start=True,
                            stop=True,
                        )
                    so = stg.tile([P, D], BF16, tag="stg", name=f"stg_{h}_{qb}")
                    nc.vector.tensor_copy(out=so[:], in_=po[:])
                    eng = nc.scalar if (h == HL - 1 and qb % 2) else nc.sync
                    eng.dma_start(out=outp[h, qb], in_=so[:])

                return f

            return [p_transpose] + [p_proj(qb) for qb in range(QB)]

        # --- attention pipeline: score head s+1 while accumulating head s;
        # head s-1's shipping pieces interleave with this head's j-loop.
        SHIP_AT = {3: 0, 5: 1, 7: 2, 9: 3, 11: 4}  # j -> piece index
        pending = []
        for s in range(HL - 1):
            avs = [av_tile(f"av_{s}_{qb}") for qb in range(QB)]
            prs = []
            for j in range(KC // 2):
                if j in SHIP_AT and SHIP_AT[j] < len(pending):
                    pending[SHIP_AT[j]]()
                if s < HL - 1:
                    hn = s + 1
                    ps = mm_tile(f"ps_{hn}_{j}")
                    for half in range(2):
                        kc = j * 2 + half
                        nc.tensor.matmul(
                            ps[:, half, :],
                            kT[:, hn, kc * P : (kc + 1) * P],
                            qT[:, hn, :],
                            start=True,
                            stop=True,
                        )
                    pr = probs_pool.tile(
                        [P, 2, 512], BF16, tag="probs", name=f"pr_{hn}_{j}"
                    )
                    nc.scalar.activation(
                        pr[:], ps[:], mybir.ActivationFunctionType.Exp, scale=SCALE
                    )
                    # gpsimd takes ~1/3 of the mask multiplies (it runs
                    # them 2x slower than DVE); no collectives to run.
                    meng = nc.gpsimd if j % 3 == 2 else nc.vector
                    meng.tensor_mul(
                        pr[:], pr[:], mask_sb[:, j * 2 : j * 2 + 2, :]
                    )
                    prs.append(pr)
                # AV for head s, k-chunks 2j, 2j+1
                for half in range(2):
                    kc = j * 2 + half
                    for qb in range(QB):
                        if s == 0:
                            lhs = pp0[:, kc, qb * P : (qb + 1) * P]
                        else:
                            lhs = cur_prs[j][:, half, qb * P : (qb + 1) * P]
                        nc.tensor.matmul(
                            avs[qb][:],
                            lhs,
                            v_sb[:, kc, s, :],
                            start=(kc == 0),
                            stop=(kc == KC - 1),
                        )
            cur_prs = prs
            num = nums.tile([P, QB, HD + 1], BF16, tag="num", name=f"num_{s}")
            for qb in range(QB):
                nc.vector.tensor_copy(out=num[:, qb, :], in_=avs[qb][:])
            pending = ship_pieces(s, num)

        # --- last head: qb-major accumulation with inline per-qb shipping
        # so the output drain starts ~6us before the final AV matmul.
        s = HL - 1
        avs3 = [av_tile(f"av3_{qb}") for qb in range(QB)]
        num3 = nums.tile([P, QB, HD + 1], BF16, tag="num", name="num3")
        PEND3 = {0: (0, 1), 1: (2,), 2: (3,), 3: (4,)}
        for qb in range(QB):
            for pi in PEND3[qb]:
                pending[pi]()
            for kc in range(KC):
                nc.tensor.matmul(
                    avs3[qb][:],
                    cur_prs[kc // 2][:, kc % 2, qb * P : (qb + 1) * P],
                    v_sb[:, kc, s, :],
                    start=(kc == 0),
                    stop=(kc == KC - 1),
                )
            nc.vector.tensor_copy(out=num3[:, qb, :], in_=avs3[qb][:])
            nc.vector.tensor_copy(
                out=dn_sb[:, s, qb : qb + 1], in_=num3[:, qb, HD : HD + 1]
            )
            pstn = mm_tile(f"pstn3_{qb}", BF16)
            nc.tensor.transpose(pstn[:, 0, 0:P], num3[:, qb, 0:HD], ident)
            nc.vector.tensor_copy(
                out=numT[:, 1, qb, :], in_=pstn[:, 0, 0:P]
            )
            po = mm_tile(f"po3_{qb}")
            for n in range(2):
                nc.tensor.matmul(
                    po[:, n, :],
                    numT[:, 1, qb, :],
                    wf_sb[:, s, n * 512 : (n + 1) * 512],
                    start=True,
                    stop=True,
                )
            so = stg.tile([P, D], BF16, tag="stg", name=f"stg3_{qb}")
            nc.vector.tensor_copy(out=so[:], in_=po[:])
            eng = nc.scalar if qb % 2 else nc.sync
            eng.dma_start(out=outp[s, qb], in_=so[:])
        nc.scalar.dma_start(
            out=dnum[s].rearrange("q p -> p q"), in_=dn_sb[:, s, :]
        )

    nc.compile()
    return nc


_NC_CACHE = None


def _get_nc():
    global _NC_CACHE
    if _NC_CACHE is None:
        _NC_CACHE = build_attention_kernel()
    return _NC_CACHE


def make_in_maps(inputs):
    BF = ml_dtypes.bfloat16
    inputs = {k: np.asarray(v) for k, v in inputs.items()}
    WqT = np.asarray(inputs["Wq"]).T.astype(BF)  # [din, dout]
    WkT = np.asarray(inputs["Wk"]).T.astype(BF)
    WvT = np.asarray(inputs["Wv"]).T.astype(BF)
    WfT = np.asarray(inputs["Wf"]).T.astype(BF)  # [dsum, dout]
    xqTs = [
        np.ascontiguousarray(inputs["inputs_q"][b].T.astype(BF)) for b in range(B)
    ]
    in_maps = []
    for c in range(NCORES):
        b, hh, kh = c // 4, (c % 4) // 2, c % 2
        sl = slice(kh * KSH, (kh + 1) * KSH)
        hs = slice(hh * HL * HD, (hh + 1) * HL * HD)
        in_maps.append(
            {
                "xqT": xqTs[b],
                "xkT": np.ascontiguousarray(inputs["inputs_k"][b, sl].T.astype(BF)),
                "xvT": np.ascontiguousarray(inputs["inputs_v"][b, sl].T.astype(BF)),
                "mskT": np.ascontiguousarray(
                    inputs["attention_mask"][b, :, sl].T.astype(BF)
                ),
                "wqT": np.ascontiguousarray(WqT[:, hs]),
                "wkT": np.ascontiguousarray(WkT[:, hs]),
                "wvT": np.ascontiguousarray(WvT[:, hs]),
                "wfT": np.ascontiguousarray(WfT[hs, :]),
            }
        )
    return in_maps


def gather_out(results):
    out = np.zeros((B, QL, D), np.float32)
    # per (batch, head-half): sum the k-half pair's unnormalized projected
    # partials and denominators per head, then divide (softmax combine).
    for b in range(B):
        for hh in range(2):
            cA, cB = b * 4 + hh * 2, b * 4 + hh * 2 + 1
            pA = np.asarray(results[cA]["outp"], np.float32).reshape(HL, QL, D)
            pB = np.asarray(results[cB]["outp"], np.float32).reshape(HL, QL, D)
            dA = np.asarray(results[cA]["dnum"], np.float32).reshape(HL, QL)
            dB = np.asarray(results[cB]["dnum"], np.float32).reshape(HL, QL)
            d = np.maximum(dA + dB, 1e-30)
            out[b] += ((pA + pB) / d[:, :, None]).sum(axis=0)
    return out


def kernel(**inputs) -> np.ndarray:
    ensure_ntff_hook()  # defensive: BASS_TRACE=1 in env would need the shim
    from concourse.bass_utils import run_bass_kernel_spmd

    nc = _get_nc()
    in_maps = make_in_maps(inputs)
    res = run_bass_kernel_spmd(nc, in_maps, list(range(NCORES)))
    return gather_out(res.results)
